# revision 4
# baseline (speedup 1.0000x reference)
"""Trainium2 Bass kernel for nn_AttentionLayer (N=2, L=2048, D=512, H=8).

reference:
    q = k = v = x.reshape(n, l, h, d)
    scores = einsum('nlhd,nshd->nhls', q, k) * (1/sqrt(d))
    W = softmax(scores, axis=-1)
    out  = einsum('nhls,nshd->nlhd', W, v).reshape(n, l, h*d)
    attn = W.mean(axis over h)  -> [n, l, s]

Sharding: 8 cores = (n in {0,1}) x (l-block in {0..3}, 512 rows each).
Per core, per head: S^T[s, l] = K^T.T @ Q^T computed via PE (fp16 inputs,
row-packed pairs of heads), exp on ScalarE (scale=1/8 folded), AV matmul
with ones-augmented V (rowsum falls out as row 64), reciprocal +
partition-broadcast, fused (P * (1/(8r))) via scalar_tensor_tensor, and
head-sum accumulated in SBUF (bf16).  Host gathers, normalizes `out` by
the rowsums, and transposes blocks back.
"""

import sys

sys.path.insert(0, "/opt/trn_rl_repo")

import numpy as np
import ml_dtypes

import concourse.bass as bass
import concourse.mybir as mybir
import concourse.tile as tile
from concourse import bacc
from concourse.bass_utils import run_bass_kernel_spmd

dt = mybir.dt
AF = mybir.ActivationFunctionType
ALU = mybir.AluOpType

N, L, D, H = 2, 2048, 512, 8
HD = D // H            # 64 head dim
LB = 512               # l-block per core
NJ = L // 128          # 16 s-tiles
NPAIR = H // 2         # 4 head pairs
BF16 = ml_dtypes.bfloat16

# number of j-indices (of 16) whose accumulate-adds run on gpsimd
ADD_GP_J = 0


def build_nc(add_gp_j=ADD_GP_J):
    nc = bacc.Bacc("TRN2", target_bir_lowering=False, debug=False,
                   num_devices=8)
    xT = nc.declare_dram_parameter("xT", [D, L], dt.float16, isOutput=False)
    xq = nc.declare_dram_parameter("xq", [D, LB], dt.float16, isOutput=False)
    vaug = nc.declare_dram_parameter("vaug", [NJ, 128, H * 65], dt.bfloat16,
                                     isOutput=False)
    attn_o = nc.declare_dram_parameter("attn_o", [NJ, 128, LB], dt.bfloat16,
                                       isOutput=True)
    o_out = nc.declare_dram_parameter("o_out", [H, 65, LB], dt.float32,
                                      isOutput=True)

    with tile.TileContext(nc) as tc:
        with (
            tc.tile_pool(name="sb_xt", bufs=2) as p_xt,
            tc.tile_pool(name="sb_xq", bufs=2) as p_xq,
            tc.tile_pool(name="sb_v", bufs=1) as p_v,
            tc.tile_pool(name="sb_P", bufs=3) as p_P,
            tc.tile_pool(name="sb_acc", bufs=1) as p_acc,
            tc.tile_pool(name="sb_st", bufs=3) as p_st,
            tc.tile_pool(name="sb_r", bufs=3) as p_r,
            tc.tile_pool(name="qk", bufs=3, space="PSUM") as p_qk,
            tc.tile_pool(name="av", bufs=2, space="PSUM") as p_av,
        ):
            # V (augmented with ones column per head), resident all kernel
            vaug_sb = []
            for j in range(NJ):
                v = p_v.tile([128, H * 65], dt.bfloat16, tag=f"v{j}")
                nc.sync.dma_start(out=v, in_=vaug[j])
                vaug_sb.append(v)

            acc = p_acc.tile([128, NJ, LB], dt.bfloat16, tag="acc")

            for t in range(NPAIR):
                # K^T rows for this head pair: xT[128t:128t+128, :]
                xt_t = p_xt.tile([128, L], dt.float16, tag="xt")
                nc.sync.dma_start(out=xt_t, in_=xT[128 * t:128 * (t + 1), :])
                xq_t = p_xq.tile([128, LB], dt.float16, tag="xq")
                nc.sync.dma_start(out=xq_t, in_=xq[128 * t:128 * (t + 1), :])

                Pt = p_P.tile([128, NJ, 2, LB], dt.bfloat16, tag="P")

                # --- QK^T (row-packed pair) + exp ---
                for j in range(NJ):
                    qk = p_qk.tile([128, 2 * LB], dt.float32, tag="qk")
                    for half in range(2):
                        nc.tensor.matmul(
                            qk[:, half * LB:(half + 1) * LB],
                            lhsT=xt_t[64 * half:64 * half + 64,
                                      128 * j:128 * (j + 1)],
                            rhs=xq_t[64 * half:64 * half + 64, :],
                            start=True, stop=True,
                            tile_position=(64 * half, 0),
                        )
                    # P = exp(S/8), both halves in one ACT op (FD=1024)
                    nc.scalar.activation(
                        out=Pt[:, j, :, :], in_=qk,
                        func=AF.Exp, scale=0.125,
                    )

                # --- per head: AV (+rowsum), recip, scale, accumulate ---
                for half in range(2):
                    h = 2 * t + half
                    av = p_av.tile([128, LB], dt.float32, tag="av")
                    for j in range(NJ):
                        nc.tensor.matmul(
                            av[0:65, :],
                            lhsT=vaug_sb[j][:, h * 65:(h + 1) * 65],
                            rhs=Pt[:, j, half, :],
                            start=(j == 0), stop=(j == NJ - 1),
                        )
                    # unnormalized out^T (64 rows) + rowsum r (row 64) -> host
                    o_st = p_st.tile([128, LB], dt.float32, tag="ost")
                    nc.vector.tensor_copy(o_st[0:65, :], av[0:65, :])
                    nc.sync.dma_start(out=o_out[h], in_=o_st[0:65, :])

                    # R8 = broadcast(0.125 / r) as bf16 [128, LB]
                    rec = p_r.tile([1, LB], dt.float32, tag="rec")
                    nc.vector.reciprocal(rec, av[64:65, :])
                    rec8 = p_r.tile([1, LB], dt.bfloat16, tag="rec8")
                    nc.vector.tensor_scalar_mul(rec8, rec, 0.125)
                    R8 = p_r.tile([128, LB], dt.bfloat16, tag="R8")
                    nc.gpsimd.partition_broadcast(R8, rec8)

                    # P_hat = P * R8 (fused); first head writes acc directly
                    srcP = Pt[:, :, half, :]                 # [128, NJ, LB]
                    R8b = R8.unsqueeze(1).broadcast_to([128, NJ, LB])
                    if t == 0 and half == 0:
                        nc.vector.scalar_tensor_tensor(
                            out=acc[:, :, :], in0=srcP, scalar=1.0,
                            in1=R8b, op0=ALU.mult, op1=ALU.mult)
                    else:
                        nc.vector.scalar_tensor_tensor(
                            out=srcP, in0=srcP, scalar=1.0,
                            in1=R8b, op0=ALU.mult, op1=ALU.mult)
                        ndve = NJ - add_gp_j
                        if ndve > 0:
                            nc.vector.tensor_add(
                                acc[:, 0:ndve, :], acc[:, 0:ndve, :],
                                srcP[:, 0:ndve, :])
                        if add_gp_j > 0:
                            nc.gpsimd.tensor_add(
                                acc[:, ndve:NJ, :], acc[:, ndve:NJ, :],
                                srcP[:, ndve:NJ, :])

            for j in range(NJ):
                nc.sync.dma_start(out=attn_o[j], in_=acc[:, j, :])

    nc.compile()
    return nc


_NC_CACHE = {}


def _get_nc():
    key = ADD_GP_J
    if key not in _NC_CACHE:
        _NC_CACHE[key] = build_nc(key)
    return _NC_CACHE[key]


def make_in_maps(x):
    x = np.asarray(x, np.float32)
    in_maps = []
    for n in range(N):
        xn = x[n]                                     # [L, D]
        xT16 = np.ascontiguousarray(xn.T).astype(np.float16)   # [D, L]
        va = np.empty((NJ, 128, H, 65), np.float32)
        va[..., :64] = xn.reshape(NJ, 128, H, HD)
        va[..., 64] = 1.0
        va16 = np.ascontiguousarray(va.reshape(NJ, 128, H * 65)).astype(BF16)
        for b in range(L // LB):
            lb = LB * b
            in_maps.append({
                "xT": xT16,
                "xq": np.ascontiguousarray(xT16[:, lb:lb + LB]),
                "vaug": va16,
            })
    return in_maps


def assemble(results):
    out = np.empty((N, L, D), np.float32)
    attn = np.empty((N, L, L), np.float32)
    for c, res in enumerate(results):
        n, b = divmod(c, L // LB)
        lb = LB * b
        o = np.asarray(res["o_out"], np.float32)      # [H, 65, LB]
        for h in range(H):
            r = o[h, 64]                              # [LB]
            out[n, lb:lb + LB, HD * h:HD * (h + 1)] = (o[h, :64] / r).T
        a = np.asarray(res["attn_o"], np.float32).reshape(L, LB)  # [s, l]
        attn[n, lb:lb + LB, :] = a.T
    return out, attn


def run(x, trace=False):
    nc = _get_nc()
    in_maps = make_in_maps(x)
    r = run_bass_kernel_spmd(nc, in_maps, core_ids=list(range(8)),
                             trace=trace)
    out, attn = assemble(r.results)
    return (out, attn), r.exec_time_ns


def kernel(input_data):
    (out, attn), _ = run(input_data, trace=False)
    return out, attn
